# revision 23
# baseline (speedup 1.0000x reference)
"""Distributed Trainium2 kernel for the audio-visual contrastive loss.

Math (reference):
    a = l2norm(audio)  (B=32, Na=512, D=768)
    v = l2norm(visual) (B=32, Nv=256, D=768)
    token_sims[b,c,n,m] = (a[b,n] . v[c,m]) / T
    clip_sims = mean_n max_m token_sims          (B, B)
    loss = mean_b -0.5*(log_softmax(clip)[b,b] + log_softmax(clip.T)[b,b])

Distribution over 8 NeuronCores:
    - audio batch is sharded 4 clips/core; visual batch is sharded 4 clips/core
      for the (normalize + transpose) prep, then AllGather'd (as bf16, d-major)
      in 2 chunks so the second chunk's collective overlaps the first chunk's
      matmuls.
    - each core computes its (4, 32) block of clip_sims:
        S[n, m] = aT[:, n].T @ vT[:, m]  accumulated over 6 d-chunks in PSUM,
        row-max over m on VectorE, column-sum over n via a ones-matmul.
    - the (4,32) blocks are AllGather'd (tiny) and every core computes the
      final scalar loss redundantly.
"""

import os
import sys

for _p in ("/opt/trn_rl_repo",):
    if _p not in sys.path:
        sys.path.insert(0, _p)

import numpy as np

import concourse.bacc as bacc
import concourse.mybir as mybir
import concourse.tile as tile

N_CORES = 8
B = 32
NA = int(os.environ.get("KERNEL_NA", "512"))
NV = 256
D = int(os.environ.get("KERNEL_D", "768"))
TEMPERATURE = 0.1
BL = B // N_CORES            # 4 clips per core
AROWS = BL * NA              # 2048 audio rows per core
VROWS = BL * NV              # 1024 visual rows per core
KD = D // 128                # 6 contraction chunks
NT_A = AROWS // 128          # 16 audio row-tiles
NT_V = VROWS // 128          # 8 visual row-tiles
G = int(os.environ.get("KERNEL_GATHER_CHUNKS", "1"))  # visual AllGather chunks
VCH = VROWS // G             # visual rows per chunk per core
CPC = BL // G                # clips per chunk per core

F32 = mybir.dt.float32
BF16 = mybir.dt.bfloat16
AX = mybir.AxisListType
ALU = mybir.AluOpType
ACT = mybir.ActivationFunctionType


def build():
    nc = bacc.Bacc("TRN2", target_bir_lowering=False, debug=False,
                   num_devices=N_CORES)
    a_in = nc.declare_dram_parameter("audio", [AROWS, D], F32, isOutput=False)
    v_in = nc.declare_dram_parameter("visual", [VROWS, D], F32, isOutput=False)
    out = nc.declare_dram_parameter("out", [1, 1], F32, isOutput=True)
    ident_dram = nc.inline_tensor(np.eye(128, dtype=np.float32), name="ident")
    rg = [list(range(N_CORES))]

    with tile.TileContext(nc) as tc:
        with (
            tc.tile_pool(name="persist", bufs=1) as pp,
            tc.tile_pool(name="work", bufs=3) as wp,
            tc.tile_pool(name="ps", bufs=8, space="PSUM") as ps,
            tc.tile_pool(name="dram", bufs=1, space="DRAM") as dp,
        ):
            # ---- constants -------------------------------------------------
            ident_f32 = pp.tile([128, 128], F32, tag="identf")
            nc.sync.dma_start(out=ident_f32[:], in_=ident_dram[:])
            ident_bf = pp.tile([128, 128], BF16, tag="identb")
            nc.scalar.copy(ident_bf[:], ident_f32[:])
            ones = pp.tile([128, 1], F32, tag="ones")
            nc.gpsimd.memset(ones[:], 1.0)

            # ---- persistent tensors ---------------------------------------
            aT = [pp.tile([128, AROWS], BF16, tag=f"aT{k}", name=f"aT{k}")
                  for k in range(KD)]
            vst = [pp.tile([128, VROWS], BF16, tag=f"vst{k}", name=f"vst{k}")
                   for k in range(KD)]
            vT = [pp.tile([128, N_CORES * VROWS], BF16, tag=f"vT{k}",
                          name=f"vT{k}") for k in range(KD)]
            mxw = (NA // 128) * 128
            mx = pp.tile([128, mxw], F32, tag="mx")

            # ---- row-tile prep: normalize rows, cast bf16, transpose ------
            dbg = set(os.environ.get("KERNEL_DEBUG_MODE", "full").split(","))

            def prep_tile(src_ap, dsts):
                raw = wp.tile([128, D], F32, tag="raw", name="raw")
                nc.sync.dma_start(out=raw[:], in_=src_ap)
                nbf = wp.tile([128, D], BF16, tag="nbf", name="nbf")
                if "nonorm" in dbg:
                    nc.scalar.copy(nbf[:], raw[:])
                else:
                    sq = wp.tile([128, D], F32, tag="sq", name="sq")
                    ss = wp.tile([128, 1], F32, tag="ss", name="ss")
                    nc.scalar.activation(sq[:], raw[:], ACT.Square,
                                         accum_out=ss[:])
                    nrm = wp.tile([128, 1], F32, tag="nrm", name="nrm")
                    nc.scalar.sqrt(nrm[:], ss[:])
                    rn = wp.tile([128, 1], F32, tag="rn", name="rn")
                    nc.vector.reciprocal(rn[:], nrm[:])
                    nc.scalar.activation(nbf[:], raw[:], ACT.Copy, bias=0.0,
                                         scale=rn[:])
                for k in range(KD):
                    dst_tile, col = dsts[k]
                    if "notrans" in dbg:
                        nc.scalar.copy(dst_tile[:, col:col + 128],
                                       nbf[:, 128 * k:128 * (k + 1)])
                    else:
                        pt = ps.tile([128, 128], BF16, tag="ps", name="pt")
                        nc.tensor.transpose(pt[:],
                                            nbf[:, 128 * k:128 * (k + 1)],
                                            ident_bf[:])
                        nc.scalar.copy(dst_tile[:, col:col + 128], pt[:])

            # visual prep first so the collectives start early
            for t in range(NT_V):
                prep_tile(v_in[t * 128:(t + 1) * 128, :],
                          [(vst[k], t * 128) for k in range(KD)])

            # ---- visual bounce + chunked AllGather ------------------------
            # bf16 data, but the bounce/gather tensors are declared f32 and
            # filled via bitcast so the collective moves f32-typed elements.
            debug_mode = set(
                os.environ.get("KERNEL_DEBUG_MODE", "full").split(","))
            vgath = []
            for g in range(G):
                vb = dp.tile([KD, 128, VCH // 2], F32, tag=f"vb{g}",
                             name=f"vb{g}")
                for k in range(KD):
                    nc.sync.dma_start(
                        out=vb[k],
                        in_=vst[k][:, g * VCH:(g + 1) * VCH].bitcast(F32))
                vg = dp.tile([N_CORES * KD, 128, VCH // 2], F32, tag=f"vg{g}",
                             name=f"vg{g}", addr_space="Shared")
                if not (debug_mode & {"nogather", "nocc"}):
                    nc.gpsimd.collective_compute(
                        "AllGather", ALU.bypass, replica_groups=rg,
                        ins=[vb[:, :, :].opt()], outs=[vg[:, :, :].opt()])
                else:
                    # debug: pretend-gather by broadcasting the local bounce
                    nc.sync.dma_start(
                        out=vg[:, :, :].rearrange("(r k) p m -> r k p m",
                                                  r=N_CORES),
                        in_=vb[None, :, :, :].broadcast_to(
                            (N_CORES, KD, 128, VCH // 2)))
                vgath.append(vg)

            # ---- audio prep ----------------------------------------------
            for t in range(NT_A):
                prep_tile(a_in[t * 128:(t + 1) * 128, :],
                          [(aT[k], t * 128) for k in range(KD)])

            # ---- load gathered visual into SBUF ---------------------------
            # vT[k] free layout: col = g*(8*VCH) + i*VCH + (j*256 + m)
            #   -> global clip c = 4*i + g*CPC + j
            for g in range(G):
                for k in range(KD):
                    for i in range(N_CORES):
                        nc.sync.dma_start(
                            out=vT[k][:, g * (N_CORES * VCH) + i * VCH:
                                      g * (N_CORES * VCH) + (i + 1) * VCH
                                      ].bitcast(F32),
                            in_=vgath[g][i * KD + k])

            # ---- main loop: S = aT.T @ vT, rowmax, accumulate -------------
            # mx col layout: nt*128 + b*32 + c  (c = global clip index)
            if "nomm" in debug_mode:
                nc.gpsimd.memset(mx[:], 0.5)
            for g in range(G if "nomm" not in debug_mode else 0):
                for b in range(BL):
                    for nt in range(NA // 128):
                        lcol = (b * (NA // 128) + nt) * 128
                        for h in range(CPC):
                            base = g * (N_CORES * VCH) + h * 2048
                            pss = [ps.tile([128, 512], F32, tag="ps",
                                           name="mm") for _ in range(4)]
                            for k in range(KD):
                                lhs = aT[k][:, lcol:lcol + 128]
                                for p in range(4):
                                    nc.tensor.matmul(
                                        pss[p][:], lhsT=lhs,
                                        rhs=vT[k][:, base + p * 512:
                                                  base + (p + 1) * 512],
                                        start=(k == 0), stop=(k == KD - 1))
                            for p in range(4):
                                f = (h * 4 + p) * 2
                                c0 = 4 * (f // CPC) + g * CPC + (f % CPC)
                                mcol = nt * 128 + b * 32 + c0
                                nc.vector.tensor_reduce(
                                    out=mx[:, mcol:mcol + 2],
                                    in_=pss[p][:].rearrange(
                                        "p (j m) -> p j m", j=2),
                                    axis=AX.X, op=ALU.max)

            # ---- column sums of row-maxes: mean over n --------------------
            pclip = ps.tile([1, mxw], F32, tag="ps", name="pclip")
            nc.tensor.matmul(pclip[:], lhsT=ones[:], rhs=mx[:],
                             start=True, stop=True)
            csum = wp.tile([1, 128], F32, tag="csum")
            nc.vector.tensor_reduce(
                out=csum[:],
                in_=pclip[:].rearrange("p (nt bc) -> p bc nt", nt=NA // 128),
                axis=AX.X, op=ALU.add)
            clip_blk = wp.tile([1, 128], F32, tag="clipblk")
            nc.scalar.mul(clip_blk[:], csum[:], 1.0 / (NA * TEMPERATURE))

            # ---- gather the (4,32) clip blocks ----------------------------
            cb = dp.tile([1, 128], F32, tag="cb", name="cb")
            nc.sync.dma_start(out=cb[:], in_=clip_blk[:])
            call = dp.tile([N_CORES, 128], F32, tag="call", name="call",
                           addr_space="Shared")
            if "nocc" not in debug_mode:
                nc.gpsimd.collective_compute(
                    "AllGather", ALU.bypass, replica_groups=rg,
                    ins=[cb[:, :].opt()], outs=[call[:, :].opt()])
            else:
                nc.sync.dma_start(
                    out=call[:, :].rearrange("(r o) c -> r o c", o=1),
                    in_=cb[None, :, :].broadcast_to((N_CORES, 1, 128)))

            # ---- final loss (computed redundantly on every core) ----------
            clip_sb = wp.tile([32, 32], F32, tag="clip")
            nc.sync.dma_start(
                out=clip_sb[:],
                in_=call[:, :].rearrange("a (b c) -> (a b) c", b=4))
            pT = ps.tile([32, 32], F32, tag="ps", name="pT")
            nc.tensor.matmul(pT[:], lhsT=clip_sb[:], rhs=ident_f32[0:32, 0:32],
                             is_transpose=True)
            clipT = wp.tile([32, 32], F32, tag="clipT")
            nc.scalar.copy(clipT[:], pT[:])

            def lse_rows(x, nm_tag):
                nm = wp.tile([32, 1], F32, tag=nm_tag + "nm", name="nm")
                nc.vector.tensor_reduce(out=nm[:], in_=x[:], axis=AX.X,
                                        op=ALU.max, negate=True)
                ex = wp.tile([32, 32], F32, tag=nm_tag + "ex", name="ex")
                es = wp.tile([32, 1], F32, tag=nm_tag + "es", name="es")
                nc.scalar.activation(ex[:], x[:], ACT.Exp, bias=nm[:],
                                     scale=1.0, accum_out=es[:])
                ln = wp.tile([32, 1], F32, tag=nm_tag + "ln", name="ln")
                nc.scalar.activation(ln[:], es[:], ACT.Ln)
                lse = wp.tile([32, 1], F32, tag=nm_tag + "lse", name="lse")
                nc.vector.tensor_sub(lse[:], ln[:], nm[:])
                return lse

            lse1 = lse_rows(clip_sb, "r")
            lse2 = lse_rows(clipT, "c")
            dsc = wp.tile([32, 32], F32, tag="dsc")
            diag = wp.tile([32, 1], F32, tag="diag")
            nc.vector.tensor_mul(dsc[:], clip_sb[:], ident_f32[0:32, 0:32])
            nc.vector.reduce_sum(out=diag[:], in_=dsc[:], axis=AX.X)
            s = wp.tile([32, 1], F32, tag="s")
            nc.vector.tensor_add(s[:], lse1[:], lse2[:])
            lb = wp.tile([32, 1], F32, tag="lb")
            nc.vector.scalar_tensor_tensor(
                out=lb[:], in0=s[:], scalar=0.5, in1=diag[:],
                op0=ALU.mult, op1=ALU.subtract)
            pl = ps.tile([1, 1], F32, tag="ps", name="pl")
            nc.tensor.matmul(pl[:], lhsT=ones[0:32, :], rhs=lb[:],
                             start=True, stop=True)
            res = wp.tile([1, 1], F32, tag="res")
            nc.scalar.mul(res[:], pl[:], 1.0 / B)
            nc.sync.dma_start(out=out[:], in_=res[:])

    nc.finalize()
    return nc


_NC_CACHE = None


def kernel(audio_feats: np.ndarray, visual_feats: np.ndarray) -> np.ndarray:
    from concourse.bass_utils import run_bass_kernel_spmd

    global _NC_CACHE
    if _NC_CACHE is None:
        _NC_CACHE = build()
    nc = _NC_CACHE

    audio = np.ascontiguousarray(audio_feats, dtype=np.float32)
    visual = np.ascontiguousarray(visual_feats, dtype=np.float32)
    in_maps = []
    for i in range(N_CORES):
        in_maps.append({
            "audio": audio[i * BL:(i + 1) * BL].reshape(AROWS, D),
            "visual": visual[i * BL:(i + 1) * BL].reshape(VROWS, D),
        })
    res = run_bass_kernel_spmd(nc, in_maps, core_ids=list(range(N_CORES)))
    val = res.results[0]["out"][0, 0]
    return np.asarray(val, dtype=np.float32)


if __name__ == "__main__":
    rng = np.random.default_rng(0)
    a = rng.standard_normal((B, NA, D)).astype(np.float32)
    v = rng.standard_normal((B, NV, D)).astype(np.float32)
    print(kernel(a, v))


# revision 26
# speedup vs baseline: 1.0777x; 1.0777x over previous
"""Distributed Trainium2 kernel for the audio-visual contrastive loss.

Math (reference):
    a = l2norm(audio)  (B=32, Na=512, D=768)
    v = l2norm(visual) (B=32, Nv=256, D=768)
    token_sims[b,c,n,m] = (a[b,n] . v[c,m]) / T
    clip_sims = mean_n max_m token_sims          (B, B)
    loss = mean_b -0.5*(log_softmax(clip)[b,b] + log_softmax(clip.T)[b,b])

Distribution over 8 NeuronCores:
    - audio batch is sharded 4 clips/core; visual batch is sharded 4 clips/core
      for the (normalize + transpose) prep, then AllGather'd (as bf16, d-major)
      in 2 chunks so the second chunk's collective overlaps the first chunk's
      matmuls.
    - each core computes its (4, 32) block of clip_sims:
        S[n, m] = aT[:, n].T @ vT[:, m]  accumulated over 6 d-chunks in PSUM,
        row-max over m on VectorE, column-sum over n via a ones-matmul.
    - the (4,32) blocks are AllGather'd (tiny) and every core computes the
      final scalar loss redundantly.
"""

import os
import sys

for _p in ("/opt/trn_rl_repo",):
    if _p not in sys.path:
        sys.path.insert(0, _p)

import numpy as np

import concourse.bacc as bacc
import concourse.mybir as mybir
import concourse.tile as tile

N_CORES = 8
B = 32
NA = int(os.environ.get("KERNEL_NA", "512"))
NV = 256
D = int(os.environ.get("KERNEL_D", "768"))
TEMPERATURE = 0.1
BL = B // N_CORES            # 4 clips per core
AROWS = BL * NA              # 2048 audio rows per core
VROWS = BL * NV              # 1024 visual rows per core
KD = D // 128                # 6 contraction chunks
NT_A = AROWS // 128          # 16 audio row-tiles
NT_V = VROWS // 128          # 8 visual row-tiles
G = int(os.environ.get("KERNEL_GATHER_CHUNKS", "1"))  # visual AllGather chunks
VCH = VROWS // G             # visual rows per chunk per core
CPC = BL // G                # clips per chunk per core

F32 = mybir.dt.float32
BF16 = mybir.dt.bfloat16
AX = mybir.AxisListType
ALU = mybir.AluOpType
ACT = mybir.ActivationFunctionType


def build():
    nc = bacc.Bacc("TRN2", target_bir_lowering=False, debug=False,
                   num_devices=N_CORES)
    a_in = nc.declare_dram_parameter("audio", [AROWS, D], F32, isOutput=False)
    v_in = nc.declare_dram_parameter("visual", [VROWS, D], F32, isOutput=False)
    out = nc.declare_dram_parameter("out", [1, 1], F32, isOutput=True)
    ident_dram = nc.inline_tensor(np.eye(128, dtype=np.float32), name="ident")
    rg = [list(range(N_CORES))]

    with tile.TileContext(nc) as tc:
        with (
            tc.tile_pool(name="persist", bufs=1) as pp,
            tc.tile_pool(name="work", bufs=3) as wp,
            tc.tile_pool(name="ps", bufs=8, space="PSUM") as ps,
            tc.tile_pool(name="dram", bufs=1, space="DRAM") as dp,
        ):
            # ---- constants -------------------------------------------------
            ident_f32 = pp.tile([128, 128], F32, tag="identf")
            nc.sync.dma_start(out=ident_f32[:], in_=ident_dram[:])
            ident_bf = pp.tile([128, 128], BF16, tag="identb")
            nc.scalar.copy(ident_bf[:], ident_f32[:])
            ones = pp.tile([128, 1], F32, tag="ones")
            nc.gpsimd.memset(ones[:], 1.0)

            # ---- persistent tensors ---------------------------------------
            aT = [pp.tile([128, AROWS], BF16, tag=f"aT{k}", name=f"aT{k}")
                  for k in range(KD)]
            vst = [pp.tile([128, VROWS], BF16, tag=f"vst{k}", name=f"vst{k}")
                   for k in range(KD)]
            vT = [pp.tile([128, N_CORES * VROWS], BF16, tag=f"vT{k}",
                          name=f"vT{k}") for k in range(KD)]
            mxw = (NA // 128) * 128
            mx = pp.tile([128, mxw], F32, tag="mx")

            # ---- row-tile prep: normalize rows, cast bf16, transpose ------
            # Row-tile prep, batched per-op so each engine runs bursts of the
            # same instruction instead of per-tile cross-engine round trips:
            #   wave of 8: DMA loads -> ACT Square(accum=ss) -> ACT sqrt ->
            #   DVE reciprocal -> ACT scaled casts -> PE transposes ->
            #   DVE psum->sbuf copies
            def prep_batch(src, t0, nb, dst_of):
                raws = []
                ssb = wp.tile([128, nb], F32, tag="ssb", name="ssb", bufs=2)
                for j in range(nb):
                    t = t0 + j
                    raw = wp.tile([128, D], F32, tag="raw", name="raw", bufs=8)
                    nc.sync.dma_start(out=raw[:],
                                      in_=src[t * 128:(t + 1) * 128, :])
                    sqs = wp.tile([128, D], F32, tag="sqs", name="sqs", bufs=2)
                    nc.scalar.activation(sqs[:], raw[:], ACT.Square,
                                         accum_out=ssb[:, j:j + 1])
                    raws.append(raw)
                nrm = wp.tile([128, nb], F32, tag="nrm", name="nrm", bufs=2)
                nc.scalar.sqrt(nrm[:], ssb[:])
                rnb = wp.tile([128, nb], F32, tag="rnb", name="rnb", bufs=2)
                nc.vector.reciprocal(rnb[:], nrm[:])
                for j in range(nb):
                    t = t0 + j
                    nbf = wp.tile([128, D], BF16, tag="nbf", name="nbf",
                                  bufs=4)
                    nc.scalar.activation(nbf[:], raws[j][:], ACT.Copy,
                                         bias=0.0, scale=rnb[:, j:j + 1])
                    for k in range(KD):
                        pt = ps.tile([128, 128], BF16, tag="ps", name="pt")
                        nc.tensor.transpose(pt[:],
                                            nbf[:, 128 * k:128 * (k + 1)],
                                            ident_bf[:])
                        dst_tile, col = dst_of(t, k)
                        nc.vector.tensor_copy(dst_tile[:, col:col + 128],
                                              pt[:])

            # visual prep first so the collectives start early
            prep_batch(v_in, 0, NT_V, lambda t, k: (vst[k], t * 128))

            # ---- visual bounce + chunked AllGather ------------------------
            # bf16 data, but the bounce/gather tensors are declared f32 and
            # filled via bitcast so the collective moves f32-typed elements.
            debug_mode = set(
                os.environ.get("KERNEL_DEBUG_MODE", "full").split(","))
            vgath = []
            for g in range(G):
                vb = dp.tile([KD, 128, VCH // 2], F32, tag=f"vb{g}",
                             name=f"vb{g}")
                for k in range(KD):
                    nc.sync.dma_start(
                        out=vb[k],
                        in_=vst[k][:, g * VCH:(g + 1) * VCH].bitcast(F32))
                vg = dp.tile([N_CORES * KD, 128, VCH // 2], F32, tag=f"vg{g}",
                             name=f"vg{g}", addr_space="Shared")
                if not (debug_mode & {"nogather", "nocc"}):
                    nc.gpsimd.collective_compute(
                        "AllGather", ALU.bypass, replica_groups=rg,
                        ins=[vb[:, :, :].opt()], outs=[vg[:, :, :].opt()])
                else:
                    # debug: pretend-gather by broadcasting the local bounce
                    nc.sync.dma_start(
                        out=vg[:, :, :].rearrange("(r k) p m -> r k p m",
                                                  r=N_CORES),
                        in_=vb[None, :, :, :].broadcast_to(
                            (N_CORES, KD, 128, VCH // 2)))
                vgath.append(vg)

            # ---- audio prep ----------------------------------------------
            for t0 in range(0, NT_A, 8):
                prep_batch(a_in, t0, min(8, NT_A - t0),
                           lambda t, k: (aT[k], t * 128))

            # ---- load gathered visual into SBUF ---------------------------
            # vT[k] free layout: col = g*(8*VCH) + i*VCH + (j*256 + m)
            #   -> global clip c = 4*i + g*CPC + j
            for g in range(G):
                for k in range(KD):
                    for i in range(N_CORES):
                        nc.sync.dma_start(
                            out=vT[k][:, g * (N_CORES * VCH) + i * VCH:
                                      g * (N_CORES * VCH) + (i + 1) * VCH
                                      ].bitcast(F32),
                            in_=vgath[g][i * KD + k])

            # ---- main loop: S = aT.T @ vT, rowmax, accumulate -------------
            # mx col layout: nt*128 + b*32 + c  (c = global clip index)
            if "nomm" in debug_mode:
                nc.gpsimd.memset(mx[:], 0.5)
            for g in range(G if "nomm" not in debug_mode else 0):
                for b in range(BL):
                    for nt in range(NA // 128):
                        lcol = (b * (NA // 128) + nt) * 128
                        for h in range(CPC):
                            base = g * (N_CORES * VCH) + h * 2048
                            pss = [ps.tile([128, 512], F32, tag="ps",
                                           name="mm") for _ in range(4)]
                            for k in range(KD):
                                lhs = aT[k][:, lcol:lcol + 128]
                                for p in range(4):
                                    nc.tensor.matmul(
                                        pss[p][:], lhsT=lhs,
                                        rhs=vT[k][:, base + p * 512:
                                                  base + (p + 1) * 512],
                                        start=(k == 0), stop=(k == KD - 1))
                            for p in range(4):
                                f = (h * 4 + p) * 2
                                c0 = 4 * (f // CPC) + g * CPC + (f % CPC)
                                mcol = nt * 128 + b * 32 + c0
                                nc.vector.tensor_reduce(
                                    out=mx[:, mcol:mcol + 2],
                                    in_=pss[p][:].rearrange(
                                        "p (j m) -> p j m", j=2),
                                    axis=AX.X, op=ALU.max)

            # ---- column sums of row-maxes: mean over n --------------------
            pclip = ps.tile([1, mxw], F32, tag="ps", name="pclip")
            nc.tensor.matmul(pclip[:], lhsT=ones[:], rhs=mx[:],
                             start=True, stop=True)
            csum = wp.tile([1, 128], F32, tag="csum")
            nc.vector.tensor_reduce(
                out=csum[:],
                in_=pclip[:].rearrange("p (nt bc) -> p bc nt", nt=NA // 128),
                axis=AX.X, op=ALU.add)
            clip_blk = wp.tile([1, 128], F32, tag="clipblk")
            nc.scalar.mul(clip_blk[:], csum[:], 1.0 / (NA * TEMPERATURE))

            # ---- gather the (4,32) clip blocks ----------------------------
            cb = dp.tile([1, 128], F32, tag="cb", name="cb")
            nc.sync.dma_start(out=cb[:], in_=clip_blk[:])
            call = dp.tile([N_CORES, 128], F32, tag="call", name="call",
                           addr_space="Shared")
            if "nocc" not in debug_mode:
                nc.gpsimd.collective_compute(
                    "AllGather", ALU.bypass, replica_groups=rg,
                    ins=[cb[:, :].opt()], outs=[call[:, :].opt()])
            else:
                nc.sync.dma_start(
                    out=call[:, :].rearrange("(r o) c -> r o c", o=1),
                    in_=cb[None, :, :].broadcast_to((N_CORES, 1, 128)))

            # ---- final loss (computed redundantly on every core) ----------
            clip_sb = wp.tile([32, 32], F32, tag="clip")
            nc.sync.dma_start(
                out=clip_sb[:],
                in_=call[:, :].rearrange("a (b c) -> (a b) c", b=4))
            pT = ps.tile([32, 32], F32, tag="ps", name="pT")
            nc.tensor.matmul(pT[:], lhsT=clip_sb[:], rhs=ident_f32[0:32, 0:32],
                             is_transpose=True)
            clipT = wp.tile([32, 32], F32, tag="clipT")
            nc.scalar.copy(clipT[:], pT[:])

            def lse_rows(x, nm_tag):
                # no max-stabilization: |clip| <= 1/T = 10, exp is safe in f32
                ex = wp.tile([32, 32], F32, tag=nm_tag + "ex", name="ex")
                es = wp.tile([32, 1], F32, tag=nm_tag + "es", name="es")
                nc.scalar.activation(ex[:], x[:], ACT.Exp, accum_out=es[:])
                lse = wp.tile([32, 1], F32, tag=nm_tag + "lse", name="lse")
                nc.scalar.activation(lse[:], es[:], ACT.Ln)
                return lse

            lse1 = lse_rows(clip_sb, "r")
            lse2 = lse_rows(clipT, "c")
            dsc = wp.tile([32, 32], F32, tag="dsc")
            diag = wp.tile([32, 1], F32, tag="diag")
            nc.vector.tensor_mul(dsc[:], clip_sb[:], ident_f32[0:32, 0:32])
            nc.vector.reduce_sum(out=diag[:], in_=dsc[:], axis=AX.X)
            s = wp.tile([32, 1], F32, tag="s")
            nc.vector.tensor_add(s[:], lse1[:], lse2[:])
            lb = wp.tile([32, 1], F32, tag="lb")
            nc.vector.scalar_tensor_tensor(
                out=lb[:], in0=s[:], scalar=0.5, in1=diag[:],
                op0=ALU.mult, op1=ALU.subtract)
            pl = ps.tile([1, 1], F32, tag="ps", name="pl")
            nc.tensor.matmul(pl[:], lhsT=ones[0:32, :], rhs=lb[:],
                             start=True, stop=True)
            res = wp.tile([1, 1], F32, tag="res")
            nc.scalar.mul(res[:], pl[:], 1.0 / B)
            nc.sync.dma_start(out=out[:], in_=res[:])

    nc.finalize()
    return nc


_NC_CACHE = None


def kernel(audio_feats: np.ndarray, visual_feats: np.ndarray) -> np.ndarray:
    from concourse.bass_utils import run_bass_kernel_spmd

    global _NC_CACHE
    if _NC_CACHE is None:
        _NC_CACHE = build()
    nc = _NC_CACHE

    audio = np.ascontiguousarray(audio_feats, dtype=np.float32)
    visual = np.ascontiguousarray(visual_feats, dtype=np.float32)
    in_maps = []
    for i in range(N_CORES):
        in_maps.append({
            "audio": audio[i * BL:(i + 1) * BL].reshape(AROWS, D),
            "visual": visual[i * BL:(i + 1) * BL].reshape(VROWS, D),
        })
    res = run_bass_kernel_spmd(nc, in_maps, core_ids=list(range(N_CORES)))
    val = res.results[0]["out"][0, 0]
    return np.asarray(val, dtype=np.float32)


if __name__ == "__main__":
    rng = np.random.default_rng(0)
    a = rng.standard_normal((B, NA, D)).astype(np.float32)
    v = rng.standard_normal((B, NV, D)).astype(np.float32)
    print(kernel(a, v))
